# revision 19
# baseline (speedup 1.0000x reference)
"""Multi-head attention kernel for Trainium2 (8 NeuronCores, SPMD).

Problem: x [4,1,2048,3], W_query/W_key/W_value [1,8,3,3] ->
ctx [4,8,2048,3] = softmax((x Wq)(x Wk)^T / sqrt(3)) @ (x Wv), returned
as a (ctx, ctx) tuple matching the reference.

Sharding: 32 (batch, head) blocks over 8 cores -> core c owns batch c//2,
heads 4*(c%2) .. +4. Each core runs an identical Bass program on its slice.

Per-core device program (S=2048, heads processed in 2 pairs):
  - ACT (exp) is the roofline: 4*2048*2048 elements at 1 elem/lane/cycle
    @1.2 GHz ~= 109us + ~260 cycles/instruction overhead. Everything is
    organized to keep ACT ~100% busy on the largest exp tiles PSUM allows
    and to keep every other engine off its critical path.
  - Host precomputes Q/K projections and their 3-way bf16 splits directly
    in the stacked [128, 2048] device layout (6 product terms x 3 dims =
    18 rows per 32-row head group), so there is no on-device setup phase.
  - PSUM: s pool = 2 x [128, 1536] f32 (banks 0-5, double buffered);
    ctx = one persistent [128, 1024] tile (banks 6-7) per chunk.
  - Per (pair, query-chunk) = 32 units of [128 keys, 512 queries], packed
    3 units per s buffer (heads interleaved kt-major):
      PE:  per unit one QK matmul [32,128]x[32,512] in the head's 32-row
           group (row tiling; concurrent units land in different banks)
      ACT: one exp per buffer (F=1536/1024) -> bf16 P tiles in SBUF
      PE:  per key tile two PV matmuls [128,7]x[128,512]: even head ->
           ctx bank 6 (col group 0), odd head -> ctx bank 7 (col group 1);
           concurrent matmuls always hit different PSUM banks (same-bank
           concurrent writes corrupt each other - verified on HW), and
           PSUM start/stop accumulation over the 16 key tiles does the
           reduction for free.
  - Normalize (dripped one piece per buffer during the next chunk): one
    DVE copy moves both heads' ctx [7+denom rows, 512] to SBUF; fused
    transpose+Wv matmuls (row groups 0/1 -> s-tile banks 1/2) produce
    [q, 3e+denom] blocks; DVE reciprocal + per-partition scalar multiply;
    DMA out. ct blocks alias into the just-consumed s buffer, so they
    need no PSUM of their own.
"""

import math

import numpy as np
import ml_dtypes

import concourse.bass as bass
import concourse.bacc as bacc
import concourse.tile as tile
from concourse import mybir
from concourse.bass_utils import run_bass_kernel_spmd

f32 = mybir.dt.float32
bf16 = mybir.dt.bfloat16
EXP = mybir.ActivationFunctionType.Exp

B, H, S, D = 4, 8, 2048, 3
NCORES = 8
HPC = H // 2           # heads per core = 4
QCH = 512              # query chunk
NQ = S // QCH          # 4
KT = 128               # key tile
NKT = S // KT          # 16
NC4 = QCH // KT        # 128-query blocks per chunk = 4
NU = 2 * NKT           # units per (pair, chunk) = 32
NBUF = (NU + 2) // 3   # s buffers per chunk = 11 (10x3 + 1x2)
SCALE = 1.0 / math.sqrt(D)

# 3-way bf16 split product terms kept for q.k (drop (2,3),(3,2),(3,3))
Q_ORDER = (0, 0, 1, 0, 2, 1)
K_ORDER = (0, 1, 0, 2, 0, 1)


def _split3_bf16(a: np.ndarray):
    """3-way bf16 split: a ~= a1 + a2 + a3, each bf16."""
    a = np.ascontiguousarray(a, dtype=np.float32)
    a1 = a.astype(ml_dtypes.bfloat16)
    r = a - a1.astype(np.float32)
    a2 = r.astype(ml_dtypes.bfloat16)
    a3 = (r - a2.astype(np.float32)).astype(ml_dtypes.bfloat16)
    return a1, a2, a3


def _build_nc():
    nc = bacc.Bacc("TRN2", target_bir_lowering=False, debug=False,
                   num_devices=NCORES)

    qstk_in = nc.dram_tensor("qstk", [128, S], bf16, kind="ExternalInput").ap()
    kstk_in = nc.dram_tensor("kstk", [128, S], bf16, kind="ExternalInput").ap()
    xo_in = nc.dram_tensor("xo", [128, NKT, 7], bf16, kind="ExternalInput").ap()
    wv7_in = nc.dram_tensor("wv7", [128, 8], f32, kind="ExternalInput").ap()
    out = nc.dram_tensor("out", [HPC, S, D], f32, kind="ExternalOutput").ap()

    with tile.TileContext(nc) as tc:
        with tc.tile_pool(name="per", bufs=1) as per, \
             tc.tile_pool(name="work", bufs=1) as work, \
             tc.tile_pool(name="spool", bufs=2, space="PSUM") as spool, \
             tc.tile_pool(name="cpool", bufs=1, space="PSUM") as cpool:
            qstk = per.tile([128, S], bf16)
            kstk = per.tile([128, S], bf16)
            xo = per.tile([128, NKT, 7], bf16)
            wv7 = per.tile([128, 8], f32)

            # ACT exp-table preload: a 1-element exp with no upstream deps
            # makes the ~2.7us ACT_TABLE_LOAD overlap the input DMAs.
            tdum = per.tile([128, 1], f32)
            tdum2 = per.tile([128, 1], f32)
            nc.gpsimd.memset(tdum, 0.0)
            nc.scalar.activation(tdum2, tdum, EXP)

            nc.sync.dma_start(out=kstk, in_=kstk_in)
            nc.sync.dma_start(out=qstk, in_=qstk_in)
            nc.gpsimd.dma_start(out=xo, in_=xo_in)
            nc.gpsimd.dma_start(out=wv7, in_=wv7_in)

            pending = []  # deferred normalize pieces; each takes the
                          # current post-exp score buffer
            rec_ctr = [0]

            def emit_buffer(p, qc, b):
                """QK matmuls for s-buffer b of chunk (p, qc)."""
                s = spool.tile([128, 3 * QCH], f32, name=f"s{p}{qc}_{b}",
                               tag="s")
                for u in range(3 * b, min(3 * b + 3, NU)):
                    t, hl = u // 2, u % 2
                    h = 2 * p + hl
                    nc.tensor.matmul(
                        s[:, (u % 3) * QCH:(u % 3 + 1) * QCH],
                        lhsT=kstk[32 * h:32 * h + 32, t * KT:(t + 1) * KT],
                        rhs=qstk[32 * h:32 * h + 32, qc * QCH:(qc + 1) * QCH],
                        start=True, stop=True,
                        tile_position=(32 * h, 0),
                    )
                return s

            def mk_copy(_ctx, _sb):
                def go(s_exp):
                    for hl in range(2):
                        nc.vector.tensor_copy(
                            _sb[32 * hl:32 * hl + 7,
                                hl * QCH:(hl + 1) * QCH],
                            _ctx[32 * hl:32 * hl + 7,
                                 hl * QCH:(hl + 1) * QCH])
                return go

            def mk_group(c4, p, _sb, _ost):
                # fused transpose + Wv contraction + denom for one
                # 128-query block, both heads of the pair, plus the
                # normalization divides. ct rows land in banks 1 (even
                # head) and 2 (odd head) of the current score buffer:
                # the two matmuls run in row groups 0/1 concurrently and
                # must hit different banks.
                def go(s_exp):
                    for hl in range(2):
                        base = (1 + hl) * QCH + 16 * c4
                        nc.tensor.matmul(
                            s_exp[:, base:base + 4],
                            lhsT=_sb[32 * hl:32 * hl + 7,
                                     hl * QCH + c4 * KT:
                                     hl * QCH + (c4 + 1) * KT],
                            rhs=wv7[32 * hl:32 * hl + 7, 4 * p:4 * p + 4],
                            start=True, stop=True,
                            tile_position=(32 * hl, 0),
                        )
                    rec_ctr[0] += 1
                    rec = work.tile([128, 2], f32, name=f"r{rec_ctr[0]}",
                                    tag="rec", bufs=3)
                    nc.vector.reciprocal(
                        rec,
                        s_exp[:, QCH + 16 * c4 + 3:2 * QCH + 16 * c4 + 4:QCH])
                    for hl in range(2):
                        base = (1 + hl) * QCH + 16 * c4
                        nc.vector.tensor_scalar_mul(
                            _ost[:, c4, hl, :],
                            s_exp[:, base:base + 3],
                            rec[:, hl:hl + 1])
                return go

            def mk_out(p, qc, hl, _ost):
                def go(s_exp):
                    h = 2 * p + hl
                    dst = bass.AP(
                        tensor=out.tensor,
                        offset=(h * S * D + qc * QCH * D),
                        ap=[[D, 128], [KT * D, NC4], [1, D]],
                    )
                    nc.sync.dma_start(out=dst, in_=_ost[:, :, hl, :])
                return go

            s_cur = emit_buffer(0, 0, 0)
            chunks = [(p, qc) for p in range(2) for qc in range(NQ)]
            for ci, (p, qc) in enumerate(chunks):
                ctx = cpool.tile([128, 2 * QCH], f32, name=f"ctx{p}{qc}",
                                 tag="ctx")
                ctxsb = work.tile([128, 2 * QCH], f32, name=f"cs{p}{qc}",
                                  tag="ctxsb", bufs=2)
                ostage = work.tile([128, NC4, 2, D], f32, name=f"ost{p}{qc}",
                                   tag="ost", bufs=2)
                ptiles = {}
                done_kt = 0
                for b in range(NBUF):
                    nu = min(3 * b + 3, NU) - 3 * b
                    fsz = nu * QCH
                    pt = work.tile([128, 3 * QCH], bf16, name=f"p{p}{qc}_{b}",
                                   tag="p", bufs=3)
                    ptiles[b] = pt
                    nc.scalar.activation(pt[:, 0:fsz], s_cur[:, 0:fsz], EXP,
                                         scale=SCALE)
                    s_exp = s_cur
                    if b + 1 < NBUF:
                        s_cur = emit_buffer(p, qc, b + 1)
                    elif ci + 1 < len(chunks):
                        s_cur = emit_buffer(*chunks[ci + 1], 0)
                    else:
                        s_cur = None
                    if pending:
                        pending.pop(0)(s_exp)
                    # PV for every key tile fully exp'd by now
                    new_kt = (3 * b + nu) // 2
                    for t in range(done_kt, new_kt):
                        for hl in range(2):
                            u = 2 * t + hl
                            psrc = ptiles[u // 3]
                            nc.tensor.matmul(
                                ctx[32 * hl:32 * hl + 7,
                                    hl * QCH:(hl + 1) * QCH],
                                lhsT=xo[:, t, :],
                                rhs=psrc[:, (u % 3) * QCH:(u % 3 + 1) * QCH],
                                start=(t == 0), stop=(t == NKT - 1),
                                tile_position=(0, 32 * hl),
                            )
                    done_kt = new_kt

                # queue this chunk's normalization/output pieces
                pending.append(mk_copy(ctx, ctxsb))
                for c4 in range(NC4):
                    pending.append(mk_group(c4, p, ctxsb, ostage))
                for hl in range(2):
                    pending.append(mk_out(p, qc, hl, ostage))

            # final drain: give pieces fresh ring slots to write into
            i = 0
            while pending:
                sx = spool.tile([128, 3 * QCH], f32, name=f"sx{i}", tag="s")
                i += 1
                pending.pop(0)(sx)

    nc.compile()
    return nc


_NC_CACHE = None


def _get_nc():
    global _NC_CACHE
    if _NC_CACHE is None:
        _NC_CACHE = _build_nc()
    return _NC_CACHE


def _make_in_maps(x, W_query, W_key, W_value):
    in_maps = []
    for c in range(NCORES):
        b = c // 2
        hp = (c % 2) * HPC
        xb = x[b, 0]                                    # [S, 3]

        qstk = np.zeros((128, S), dtype=ml_dtypes.bfloat16)
        kstk = np.zeros((128, S), dtype=ml_dtypes.bfloat16)
        for h in range(HPC):
            Qh = (xb @ W_query[0, hp + h]).T            # [3, S]
            Kh = (xb @ W_key[0, hp + h]).T
            qp = _split3_bf16(Qh)
            kp = _split3_bf16(Kh)
            for t6 in range(6):
                r = 32 * h + 3 * t6
                qstk[r:r + 3] = qp[Q_ORDER[t6]]
                kstk[r:r + 3] = kp[K_ORDER[t6]]

        # xo[p, t, :] = [x_hi(3) | x_lo(3) | 1] at position t*128+p
        xh = xb.astype(ml_dtypes.bfloat16)
        xl = (xb - xh.astype(np.float32)).astype(ml_dtypes.bfloat16)
        xo = np.concatenate(
            [xh, xl, np.ones((S, 1), ml_dtypes.bfloat16)], axis=1)
        xo = np.ascontiguousarray(
            xo.reshape(NKT, 128, 7).transpose(1, 0, 2))

        # wv7 block for head 2p+hl at partitions 32*hl, columns 4p:
        # rows [Wv; Wv; denom-selector]
        wv7 = np.zeros((128, 8), np.float32)
        for h in range(HPC):
            Wv = W_value[0, hp + h]                     # [3, 3]
            pb = 32 * (h % 2)
            wc = 4 * (h // 2)
            wv7[pb + 0:pb + 3, wc:wc + 3] = Wv
            wv7[pb + 3:pb + 6, wc:wc + 3] = Wv
            wv7[pb + 6, wc + 3] = 1.0

        in_maps.append({
            "qstk": qstk,
            "kstk": kstk,
            "xo": xo,
            "wv7": wv7,
        })
    return in_maps


def kernel(x, W_query, W_key, W_value, _trace=False, _tmpdir=None):
    x = np.asarray(x, dtype=np.float32)
    W_query = np.asarray(W_query, dtype=np.float32)
    W_key = np.asarray(W_key, dtype=np.float32)
    W_value = np.asarray(W_value, dtype=np.float32)

    nc = _get_nc()
    res = run_bass_kernel_spmd(
        nc,
        _make_in_maps(x, W_query, W_key, W_value),
        core_ids=list(range(NCORES)),
        trace=_trace,
        tmpdir=_tmpdir,
    )
    full = np.empty((B, H, S, D), dtype=np.float32)
    for c in range(NCORES):
        b = c // 2
        hp = (c % 2) * HPC
        full[b, hp:hp + HPC] = res.results[c]["out"]
    if _trace:
        kernel._last_results = res
    return (full, full)


# revision 20
# speedup vs baseline: 1.1902x; 1.1902x over previous
"""Multi-head attention kernel for Trainium2 (8 NeuronCores, SPMD).

Problem: x [4,1,2048,3], W_query/W_key/W_value [1,8,3,3] ->
ctx [4,8,2048,3] = softmax((x Wq)(x Wk)^T / sqrt(3)) @ (x Wv), returned
as a (ctx, ctx) tuple matching the reference.

Sharding: 32 (batch, head) blocks over 8 cores -> core c owns batch c//2,
heads 4*(c%2) .. +4. Each core runs an identical Bass program on its slice.

Per-core device program (S=2048, heads processed in 2 pairs):
  - ACT (exp) is the roofline: 4*2048*2048 elements at 1 elem/lane/cycle
    @1.2 GHz ~= 109us + ~260 cycles/instruction overhead. Everything is
    organized to keep ACT ~100% busy on the largest exp tiles PSUM allows
    and to keep every other engine off its critical path.
  - Host precomputes Q/K projections and their 3-way bf16 splits directly
    in the stacked [128, 2048] device layout (6 product terms x 3 dims =
    18 rows per 32-row head group), so there is no on-device setup phase.
  - PSUM: s pool = 2 x [128, 1536] f32 (banks 0-5, double buffered);
    ctx = one persistent [128, 1024] tile (banks 6-7) per chunk.
  - Per (pair, query-chunk) = 32 units of [128 keys, 512 queries], packed
    3 units per s buffer (heads interleaved kt-major):
      PE:  per unit one QK matmul [32,128]x[32,512] in the head's 32-row
           group (row tiling; concurrent units land in different banks)
      ACT: one exp per buffer (F=1536/1024) -> bf16 P tiles in SBUF
      PE:  per key tile two PV matmuls [128,7]x[128,512]: even head ->
           ctx bank 6 (col group 0), odd head -> ctx bank 7 (col group 1);
           concurrent matmuls always hit different PSUM banks (same-bank
           concurrent writes corrupt each other - verified on HW), and
           PSUM start/stop accumulation over the 16 key tiles does the
           reduction for free.
  - Normalize (dripped one piece per buffer during the next chunk): one
    DVE copy moves both heads' ctx [7+denom rows, 512] to SBUF; fused
    transpose+Wv matmuls (row groups 0/1 -> s-tile banks 1/2) produce
    [q, 3e+denom] blocks; DVE reciprocal + per-partition scalar multiply;
    DMA out. ct blocks alias into the just-consumed s buffer, so they
    need no PSUM of their own.
"""

import math

import numpy as np
import ml_dtypes

import concourse.bass as bass
import concourse.bacc as bacc
import concourse.tile as tile
from concourse import mybir
from concourse.bass_utils import run_bass_kernel_spmd

f32 = mybir.dt.float32
f32r = mybir.dt.float32r
bf16 = mybir.dt.bfloat16
EXP = mybir.ActivationFunctionType.Exp

B, H, S, D = 4, 8, 2048, 3
NCORES = 8
HPC = H // 2           # heads per core = 4
QCH = 512              # query chunk
NQ = S // QCH          # 4
KT = 128               # key tile
NKT = S // KT          # 16
NC4 = QCH // KT        # 128-query blocks per chunk = 4
NU = 2 * NKT           # units per (pair, chunk) = 32
NBUF = (NU + 2) // 3   # s buffers per chunk = 11 (10x3 + 1x2)
SCALE = 1.0 / math.sqrt(D)

# 3-way bf16 split product terms kept for q.k (drop (2,3),(3,2),(3,3))
Q_ORDER = (0, 0, 1, 0, 2, 1)
K_ORDER = (0, 1, 0, 2, 0, 1)


def _split_hi_lo(a: np.ndarray):
    """Exact split a = hi + lo with both parts f32r-representable
    (11-bit mantissa, round-to-nearest with carry)."""
    a = np.ascontiguousarray(a, dtype=np.float32)
    u = a.view(np.uint32)
    r = (u + np.uint32(0x7FF) + ((u >> np.uint32(12)) & np.uint32(1))) & np.uint32(
        0xFFFFF000
    )
    hi = r.view(np.float32)
    lo = (a - hi).astype(np.float32)
    return hi, lo


def _split3_bf16(a: np.ndarray):
    """3-way bf16 split: a ~= a1 + a2 + a3, each bf16."""
    a = np.ascontiguousarray(a, dtype=np.float32)
    a1 = a.astype(ml_dtypes.bfloat16)
    r = a - a1.astype(np.float32)
    a2 = r.astype(ml_dtypes.bfloat16)
    a3 = (r - a2.astype(np.float32)).astype(ml_dtypes.bfloat16)
    return a1, a2, a3


def _build_nc():
    nc = bacc.Bacc("TRN2", target_bir_lowering=False, debug=False,
                   num_devices=NCORES)

    qstk_in = nc.dram_tensor("qstk", [128, S], bf16, kind="ExternalInput").ap()
    kstk_in = nc.dram_tensor("kstk", [128, S], bf16, kind="ExternalInput").ap()
    xo_in = nc.dram_tensor("xo", [128, NKT, 7], f32r, kind="ExternalInput").ap()
    wv7_in = nc.dram_tensor("wv7", [128, 16], bf16, kind="ExternalInput").ap()
    out = nc.dram_tensor("out", [HPC, S, D], f32, kind="ExternalOutput").ap()

    with tile.TileContext(nc) as tc:
        with tc.tile_pool(name="per", bufs=1) as per, \
             tc.tile_pool(name="work", bufs=1) as work, \
             tc.tile_pool(name="spool", bufs=2, space="PSUM") as spool, \
             tc.tile_pool(name="cpool", bufs=1, space="PSUM") as cpool:
            qstk = per.tile([128, S], bf16)
            kstk = per.tile([128, S], bf16)
            xo = per.tile([128, NKT, 7], f32r)
            wv7 = per.tile([128, 16], bf16)

            # ACT exp-table preload: a 1-element exp with no upstream deps
            # makes the ~2.7us ACT_TABLE_LOAD overlap the input DMAs.
            tdum = per.tile([128, 1], f32)
            tdum2 = per.tile([128, 1], f32)
            nc.gpsimd.memset(tdum, 0.0)
            nc.scalar.activation(tdum2, tdum, EXP)

            nc.sync.dma_start(out=kstk, in_=kstk_in)
            nc.sync.dma_start(out=qstk, in_=qstk_in)
            nc.gpsimd.dma_start(out=xo, in_=xo_in)
            nc.gpsimd.dma_start(out=wv7, in_=wv7_in)

            pending = []  # deferred normalize pieces; each takes the
                          # current post-exp score buffer
            rec_ctr = [0]

            def emit_buffer(p, qc, b):
                """QK matmuls for s-buffer b of chunk (p, qc)."""
                s = spool.tile([128, 3 * QCH], f32, name=f"s{p}{qc}_{b}",
                               tag="s")
                for u in range(3 * b, min(3 * b + 3, NU)):
                    t, hl = u // 2, u % 2
                    h = 2 * p + hl
                    nc.tensor.matmul(
                        s[:, (u % 3) * QCH:(u % 3 + 1) * QCH],
                        lhsT=kstk[32 * h:32 * h + 32, t * KT:(t + 1) * KT],
                        rhs=qstk[32 * h:32 * h + 32, qc * QCH:(qc + 1) * QCH],
                        start=True, stop=True,
                        tile_position=(32 * h, 0),
                    )
                return s

            def mk_copy(_ctx, _sb):
                def go(s_exp):
                    for hl in range(2):
                        nc.vector.tensor_copy(
                            _sb[0:7, hl * QCH:(hl + 1) * QCH],
                            _ctx[0:7, hl * QCH:(hl + 1) * QCH])
                return go

            def mk_group(c4, p, _sb, _ost):
                # fused transpose + Wv contraction + denom for one
                # 128-query block, both heads of the pair, plus the
                # normalization divides. ct rows land in banks 1 (even
                # head) and 2 (odd head) of the current score buffer:
                # the two matmuls run in row groups 0/1 concurrently and
                # must hit different banks.
                def go(s_exp):
                    for hl in range(2):
                        base = (1 + hl) * QCH + 16 * c4
                        nc.tensor.matmul(
                            s_exp[:, base:base + 4],
                            lhsT=_sb[0:7,
                                     hl * QCH + c4 * KT:
                                     hl * QCH + (c4 + 1) * KT],
                            rhs=wv7[0:7, 8 * p + 4 * hl:8 * p + 4 * hl + 4],
                            start=True, stop=True,
                            tile_position=(0, 0),
                        )
                    rec_ctr[0] += 1
                    rec = work.tile([128, 2], f32, name=f"r{rec_ctr[0]}",
                                    tag="rec", bufs=3)
                    nc.vector.reciprocal(
                        rec,
                        s_exp[:, QCH + 16 * c4 + 3:2 * QCH + 16 * c4 + 4:QCH])
                    for hl in range(2):
                        base = (1 + hl) * QCH + 16 * c4
                        nc.vector.tensor_scalar_mul(
                            _ost[:, c4, hl, :],
                            s_exp[:, base:base + 3],
                            rec[:, hl:hl + 1])
                return go

            def mk_out(p, qc, hl, _ost):
                def go(s_exp):
                    h = 2 * p + hl
                    dst = bass.AP(
                        tensor=out.tensor,
                        offset=(h * S * D + qc * QCH * D),
                        ap=[[D, 128], [KT * D, NC4], [1, D]],
                    )
                    nc.sync.dma_start(out=dst, in_=_ost[:, :, hl, :])
                return go

            s_cur = emit_buffer(0, 0, 0)
            chunks = [(p, qc) for p in range(2) for qc in range(NQ)]
            for ci, (p, qc) in enumerate(chunks):
                ctx = cpool.tile([128, 2 * QCH], f32, name=f"ctx{p}{qc}",
                                 tag="ctx")
                ctxsb = work.tile([128, 2 * QCH], bf16, name=f"cs{p}{qc}",
                                  tag="ctxsb", bufs=2)
                ostage = work.tile([128, NC4, 2, D], f32, name=f"ost{p}{qc}",
                                   tag="ost", bufs=2)
                ptiles = {}
                done_kt = 0
                for b in range(NBUF):
                    nu = min(3 * b + 3, NU) - 3 * b
                    fsz = nu * QCH
                    pt = work.tile([128, 3 * QCH], f32r, name=f"p{p}{qc}_{b}",
                                   tag="p", bufs=3)
                    ptiles[b] = pt
                    nc.scalar.activation(pt[:, 0:fsz], s_cur[:, 0:fsz], EXP,
                                         scale=SCALE)
                    s_exp = s_cur
                    if b + 1 < NBUF:
                        s_cur = emit_buffer(p, qc, b + 1)
                    elif ci + 1 < len(chunks):
                        s_cur = emit_buffer(*chunks[ci + 1], 0)
                    else:
                        s_cur = None
                    if pending:
                        pending.pop(0)(s_exp)
                    # PV for every key tile fully exp'd by now
                    new_kt = (3 * b + nu) // 2
                    for t in range(done_kt, new_kt):
                        for hl in range(2):
                            u = 2 * t + hl
                            psrc = ptiles[u // 3]
                            nc.tensor.matmul(
                                ctx[0:7, hl * QCH:(hl + 1) * QCH],
                                lhsT=xo[:, t, :],
                                rhs=psrc[:, (u % 3) * QCH:(u % 3 + 1) * QCH],
                                start=(t == 0), stop=(t == NKT - 1),
                                tile_position=(0, 0),
                            )
                    done_kt = new_kt

                # queue this chunk's normalization/output pieces
                pending.append(mk_copy(ctx, ctxsb))
                for c4 in range(NC4):
                    pending.append(mk_group(c4, p, ctxsb, ostage))
                for hl in range(2):
                    pending.append(mk_out(p, qc, hl, ostage))

            # final drain: give pieces fresh ring slots to write into
            i = 0
            while pending:
                sx = spool.tile([128, 3 * QCH], f32, name=f"sx{i}", tag="s")
                i += 1
                pending.pop(0)(sx)

    nc.compile()
    return nc


_NC_CACHE = None


def _get_nc():
    global _NC_CACHE
    if _NC_CACHE is None:
        _NC_CACHE = _build_nc()
    return _NC_CACHE


def _make_in_maps(x, W_query, W_key, W_value):
    in_maps = []
    for c in range(NCORES):
        b = c // 2
        hp = (c % 2) * HPC
        xb = x[b, 0]                                    # [S, 3]

        qstk = np.zeros((128, S), dtype=ml_dtypes.bfloat16)
        kstk = np.zeros((128, S), dtype=ml_dtypes.bfloat16)
        for h in range(HPC):
            Qh = (xb @ W_query[0, hp + h]).T            # [3, S]
            Kh = (xb @ W_key[0, hp + h]).T
            qp = _split3_bf16(Qh)
            kp = _split3_bf16(Kh)
            for t6 in range(6):
                r = 32 * h + 3 * t6
                qstk[r:r + 3] = qp[Q_ORDER[t6]]
                kstk[r:r + 3] = kp[K_ORDER[t6]]

        # xo[p, t, :] = [x_hi(3) | x_lo(3) | 1] at position t*128+p
        xh, xl = _split_hi_lo(xb)
        xo = np.concatenate([xh, xl, np.ones((S, 1), np.float32)], axis=1)
        xo = np.ascontiguousarray(
            xo.reshape(NKT, 128, 7).transpose(1, 0, 2))

        # wv7 block for head 2p+hl at partitions 0:7, columns 8p+4hl:
        # rows [Wv; Wv; denom-selector]
        wv7 = np.zeros((128, 16), ml_dtypes.bfloat16)
        for h in range(HPC):
            Wv = W_value[0, hp + h]                     # [3, 3]
            wc = 8 * (h // 2) + 4 * (h % 2)
            wv7[0:3, wc:wc + 3] = Wv
            wv7[3:6, wc:wc + 3] = Wv
            wv7[6, wc + 3] = 1.0

        in_maps.append({
            "qstk": qstk,
            "kstk": kstk,
            "xo": xo,
            "wv7": wv7,
        })
    return in_maps


def kernel(x, W_query, W_key, W_value, _trace=False, _tmpdir=None):
    x = np.asarray(x, dtype=np.float32)
    W_query = np.asarray(W_query, dtype=np.float32)
    W_key = np.asarray(W_key, dtype=np.float32)
    W_value = np.asarray(W_value, dtype=np.float32)

    nc = _get_nc()
    res = run_bass_kernel_spmd(
        nc,
        _make_in_maps(x, W_query, W_key, W_value),
        core_ids=list(range(NCORES)),
        trace=_trace,
        tmpdir=_tmpdir,
    )
    full = np.empty((B, H, S, D), dtype=np.float32)
    for c in range(NCORES):
        b = c // 2
        hp = (c % 2) * HPC
        full[b, hp:hp + HPC] = res.results[c]["out"]
    if _trace:
        kernel._last_results = res
    return (full, full)


# revision 21
# speedup vs baseline: 1.4118x; 1.1861x over previous
"""Multi-head attention kernel for Trainium2 (8 NeuronCores, SPMD).

Problem: x [4,1,2048,3], W_query/W_key/W_value [1,8,3,3] ->
ctx [4,8,2048,3] = softmax((x Wq)(x Wk)^T / sqrt(3)) @ (x Wv), returned
as a (ctx, ctx) tuple matching the reference.

Sharding: 32 (batch, head) blocks over 8 cores -> core c owns batch c//2,
heads 4*(c%2) .. +4. Each core runs an identical Bass program on its slice.

Per-core device program (S=2048, heads processed in 2 pairs):
  - ACT (exp) is the roofline: 4*2048*2048 elements at 1 elem/lane/cycle
    @1.2 GHz ~= 109us + ~260 cycles/instruction overhead. Everything is
    organized to keep ACT ~100% busy on the largest exp tiles PSUM allows
    and to keep every other engine off its critical path.
  - Host precomputes Q/K projections and their 3-way bf16 splits directly
    in the stacked [128, 2048] device layout (6 product terms x 3 dims =
    18 rows per 32-row head group), so there is no on-device setup phase.
  - PSUM: s pool = 2 x [128, 1536] f32 (banks 0-5, double buffered);
    ctx = one persistent [128, 1024] tile (banks 6-7) per chunk.
  - Per (pair, query-chunk) = 32 units of [128 keys, 512 queries], packed
    3 units per s buffer (heads interleaved kt-major):
      PE:  per unit one QK matmul [32,128]x[32,512] in the head's 32-row
           group (row tiling; concurrent units land in different banks)
      ACT: one exp per buffer (F=1536/1024) -> bf16 P tiles in SBUF
      PE:  per key tile two PV matmuls [128,7]x[128,512]: even head ->
           ctx bank 6 (col group 0), odd head -> ctx bank 7 (col group 1);
           concurrent matmuls always hit different PSUM banks (same-bank
           concurrent writes corrupt each other - verified on HW), and
           PSUM start/stop accumulation over the 16 key tiles does the
           reduction for free.
  - Normalize (dripped one piece per buffer during the next chunk): one
    DVE copy moves both heads' ctx [7+denom rows, 512] to SBUF; fused
    transpose+Wv matmuls (row groups 0/1 -> s-tile banks 1/2) produce
    [q, 3e+denom] blocks; DVE reciprocal + per-partition scalar multiply;
    DMA out. ct blocks alias into the just-consumed s buffer, so they
    need no PSUM of their own.
"""

import math

import numpy as np
import ml_dtypes

import concourse.bass as bass
import concourse.bacc as bacc
import concourse.tile as tile
from concourse import mybir
from concourse.bass_utils import run_bass_kernel_spmd

f32 = mybir.dt.float32
f32r = mybir.dt.float32r
bf16 = mybir.dt.bfloat16
EXP = mybir.ActivationFunctionType.Exp

B, H, S, D = 4, 8, 2048, 3
NCORES = 8
HPC = H // 2           # heads per core = 4
QCH = 512              # query chunk
NQ = S // QCH          # 4
KT = 128               # key tile
NKT = S // KT          # 16
NC4 = QCH // KT        # 128-query blocks per chunk = 4
NU = 2 * NKT           # units per (pair, chunk) = 32
NBUF = (NU + 2) // 3   # s buffers per chunk = 11 (10x3 + 1x2)
SCALE = 1.0 / math.sqrt(D)

# 3-way bf16 split product terms kept for q.k (drop (2,3),(3,2),(3,3))
Q_ORDER = (0, 0, 1, 0, 2, 1)
K_ORDER = (0, 1, 0, 2, 0, 1)


def _split_hi_lo(a: np.ndarray):
    """Exact split a = hi + lo with both parts f32r-representable
    (11-bit mantissa, round-to-nearest with carry)."""
    a = np.ascontiguousarray(a, dtype=np.float32)
    u = a.view(np.uint32)
    r = (u + np.uint32(0x7FF) + ((u >> np.uint32(12)) & np.uint32(1))) & np.uint32(
        0xFFFFF000
    )
    hi = r.view(np.float32)
    lo = (a - hi).astype(np.float32)
    return hi, lo


def _split3_bf16(a: np.ndarray):
    """3-way bf16 split: a ~= a1 + a2 + a3, each bf16."""
    a = np.ascontiguousarray(a, dtype=np.float32)
    a1 = a.astype(ml_dtypes.bfloat16)
    r = a - a1.astype(np.float32)
    a2 = r.astype(ml_dtypes.bfloat16)
    a3 = (r - a2.astype(np.float32)).astype(ml_dtypes.bfloat16)
    return a1, a2, a3


def _build_nc():
    nc = bacc.Bacc("TRN2", target_bir_lowering=False, debug=False,
                   num_devices=NCORES)

    qstk_in = nc.dram_tensor("qstk", [128, S], bf16, kind="ExternalInput").ap()
    kstk_in = nc.dram_tensor("kstk", [128, S], bf16, kind="ExternalInput").ap()
    xo_in = nc.dram_tensor("xo", [128, NKT, 7], f32r, kind="ExternalInput").ap()
    wv7_in = nc.dram_tensor("wv7", [128, 16], bf16, kind="ExternalInput").ap()
    out = nc.dram_tensor("out", [HPC, S, D], f32, kind="ExternalOutput").ap()

    with tile.TileContext(nc) as tc:
        with tc.tile_pool(name="per", bufs=1) as per, \
             tc.tile_pool(name="work", bufs=1) as work, \
             tc.tile_pool(name="spool", bufs=2, space="PSUM") as spool, \
             tc.tile_pool(name="cpool", bufs=1, space="PSUM") as cpool:
            qstk = per.tile([128, S], bf16)
            kstk = per.tile([128, S], bf16)
            xo = per.tile([128, NKT, 7], f32r)
            wv7 = per.tile([128, 16], bf16)

            # ACT exp-table preload: a 1-element exp with no upstream deps
            # makes the ~2.7us ACT_TABLE_LOAD overlap the input DMAs.
            tdum = per.tile([128, 1], f32)
            tdum2 = per.tile([128, 1], f32)
            nc.gpsimd.memset(tdum, 0.0)
            nc.scalar.activation(tdum2, tdum, EXP)

            # pair-0 head rows first so the first QK can start ~1.5us in;
            # the rest streams behind it on both queues
            nc.sync.dma_start(out=kstk[0:64, :], in_=kstk_in[0:64, :])
            nc.sync.dma_start(out=qstk[0:64, 0:QCH], in_=qstk_in[0:64, 0:QCH])
            nc.gpsimd.dma_start(out=wv7, in_=wv7_in)
            nc.gpsimd.dma_start(out=xo, in_=xo_in)
            nc.sync.dma_start(out=qstk[0:64, QCH:], in_=qstk_in[0:64, QCH:])
            nc.gpsimd.dma_start(out=kstk[64:128, :], in_=kstk_in[64:128, :])
            nc.sync.dma_start(out=qstk[64:128, :], in_=qstk_in[64:128, :])

            pending = []  # deferred normalize pieces; each takes the
                          # current post-exp score buffer
            rec_ctr = [0]

            def emit_buffer(p, qc, b):
                """QK matmuls for s-buffer b of chunk (p, qc)."""
                s = spool.tile([128, 3 * QCH], f32, name=f"s{p}{qc}_{b}",
                               tag="s")
                for u in range(3 * b, min(3 * b + 3, NU)):
                    t, hl = u // 2, u % 2
                    h = 2 * p + hl
                    nc.tensor.matmul(
                        s[:, (u % 3) * QCH:(u % 3 + 1) * QCH],
                        lhsT=kstk[32 * h:32 * h + 32, t * KT:(t + 1) * KT],
                        rhs=qstk[32 * h:32 * h + 32, qc * QCH:(qc + 1) * QCH],
                        start=True, stop=True,
                        tile_position=(32 * h, 0),
                    )
                return s

            def mk_copy(_ctx, _sb):
                def go(s_exp):
                    for hl in range(2):
                        nc.vector.tensor_copy(
                            _sb[0:7, hl * QCH:(hl + 1) * QCH],
                            _ctx[0:7, hl * QCH:(hl + 1) * QCH])
                return go

            def mk_group(c4, p, _sb, _ost):
                # fused transpose + Wv contraction + denom for one
                # 128-query block, both heads of the pair, plus the
                # normalization divides. ct rows land in banks 1 (even
                # head) and 2 (odd head) of the current score buffer:
                # the two matmuls run in row groups 0/1 concurrently and
                # must hit different banks.
                def go(s_exp):
                    for hl in range(2):
                        base = (1 + hl) * QCH + 16 * c4
                        nc.tensor.matmul(
                            s_exp[:, base:base + 4],
                            lhsT=_sb[0:7,
                                     hl * QCH + c4 * KT:
                                     hl * QCH + (c4 + 1) * KT],
                            rhs=wv7[0:7, 8 * p + 4 * hl:8 * p + 4 * hl + 4],
                            start=True, stop=True,
                            tile_position=(0, 0),
                        )
                    rec_ctr[0] += 1
                    rec = work.tile([128, 2], f32, name=f"r{rec_ctr[0]}",
                                    tag="rec", bufs=3)
                    nc.vector.reciprocal(
                        rec,
                        s_exp[:, QCH + 16 * c4 + 3:2 * QCH + 16 * c4 + 4:QCH])
                    for hl in range(2):
                        base = (1 + hl) * QCH + 16 * c4
                        nc.vector.tensor_scalar_mul(
                            _ost[:, c4, hl, :],
                            s_exp[:, base:base + 3],
                            rec[:, hl:hl + 1])
                go.needs_s = True
                return go

            def mk_out(p, qc, hl, _ost):
                def go(s_exp):
                    h = 2 * p + hl
                    dst = bass.AP(
                        tensor=out.tensor,
                        offset=(h * S * D + qc * QCH * D),
                        ap=[[D, 128], [KT * D, NC4], [1, D]],
                    )
                    nc.sync.dma_start(out=dst, in_=_ost[:, :, hl, :])
                return go

            s_cur = emit_buffer(0, 0, 0)
            chunks = [(p, qc) for p in range(2) for qc in range(NQ)]
            for ci, (p, qc) in enumerate(chunks):
                ctx = cpool.tile([128, 2 * QCH], f32, name=f"ctx{p}{qc}",
                                 tag="ctx")
                ctxsb = work.tile([128, 2 * QCH], bf16, name=f"cs{p}{qc}",
                                  tag="ctxsb", bufs=2)
                ostage = work.tile([128, NC4, 2, D], f32, name=f"ost{p}{qc}",
                                   tag="ost", bufs=2)
                ptiles = {}
                done_kt = 0
                for b in range(NBUF):
                    nu = min(3 * b + 3, NU) - 3 * b
                    fsz = nu * QCH
                    pt = work.tile([128, 3 * QCH], f32r, name=f"p{p}{qc}_{b}",
                                   tag="p", bufs=3)
                    ptiles[b] = pt
                    nc.scalar.activation(pt[:, 0:fsz], s_cur[:, 0:fsz], EXP,
                                         scale=SCALE)
                    s_exp = s_cur
                    if b + 1 < NBUF:
                        s_cur = emit_buffer(p, qc, b + 1)
                    elif ci + 1 < len(chunks):
                        s_cur = emit_buffer(*chunks[ci + 1], 0)
                    else:
                        s_cur = None
                    if pending:
                        pending.pop(0)(s_exp)
                    # PV for every key tile fully exp'd by now
                    new_kt = (3 * b + nu) // 2
                    for t in range(done_kt, new_kt):
                        for hl in range(2):
                            u = 2 * t + hl
                            psrc = ptiles[u // 3]
                            nc.tensor.matmul(
                                ctx[0:7, hl * QCH:(hl + 1) * QCH],
                                lhsT=xo[:, t, :],
                                rhs=psrc[:, (u % 3) * QCH:(u % 3 + 1) * QCH],
                                start=(t == 0), stop=(t == NKT - 1),
                                tile_position=(0, 0),
                            )
                    done_kt = new_kt

                # queue this chunk's normalization/output pieces
                pending.append(mk_copy(ctx, ctxsb))
                for c4 in range(NC4):
                    pending.append(mk_group(c4, p, ctxsb, ostage))
                for hl in range(2):
                    pending.append(mk_out(p, qc, hl, ostage))

            # final drain: all group pieces share one fresh ring slot
            # (their ct blocks occupy disjoint columns)
            sx = spool.tile([128, 3 * QCH], f32, name="sx", tag="s")
            while pending:
                pending.pop(0)(sx)

    nc.compile()
    return nc


_NC_CACHE = None


def _get_nc():
    global _NC_CACHE
    if _NC_CACHE is None:
        _NC_CACHE = _build_nc()
    return _NC_CACHE


def _make_in_maps(x, W_query, W_key, W_value):
    in_maps = []
    for c in range(NCORES):
        b = c // 2
        hp = (c % 2) * HPC
        xb = x[b, 0]                                    # [S, 3]

        qstk = np.zeros((128, S), dtype=ml_dtypes.bfloat16)
        kstk = np.zeros((128, S), dtype=ml_dtypes.bfloat16)
        for h in range(HPC):
            Qh = (xb @ W_query[0, hp + h]).T            # [3, S]
            Kh = (xb @ W_key[0, hp + h]).T
            qp = _split3_bf16(Qh)
            kp = _split3_bf16(Kh)
            for t6 in range(6):
                r = 32 * h + 3 * t6
                qstk[r:r + 3] = qp[Q_ORDER[t6]]
                kstk[r:r + 3] = kp[K_ORDER[t6]]

        # xo[p, t, :] = [x_hi(3) | x_lo(3) | 1] at position t*128+p
        xh, xl = _split_hi_lo(xb)
        xo = np.concatenate([xh, xl, np.ones((S, 1), np.float32)], axis=1)
        xo = np.ascontiguousarray(
            xo.reshape(NKT, 128, 7).transpose(1, 0, 2))

        # wv7 block for head 2p+hl at partitions 0:7, columns 8p+4hl:
        # rows [Wv; Wv; denom-selector]
        wv7 = np.zeros((128, 16), ml_dtypes.bfloat16)
        for h in range(HPC):
            Wv = W_value[0, hp + h]                     # [3, 3]
            wc = 8 * (h // 2) + 4 * (h % 2)
            wv7[0:3, wc:wc + 3] = Wv
            wv7[3:6, wc:wc + 3] = Wv
            wv7[6, wc + 3] = 1.0

        in_maps.append({
            "qstk": qstk,
            "kstk": kstk,
            "xo": xo,
            "wv7": wv7,
        })
    return in_maps


def kernel(x, W_query, W_key, W_value, _trace=False, _tmpdir=None):
    x = np.asarray(x, dtype=np.float32)
    W_query = np.asarray(W_query, dtype=np.float32)
    W_key = np.asarray(W_key, dtype=np.float32)
    W_value = np.asarray(W_value, dtype=np.float32)

    nc = _get_nc()
    res = run_bass_kernel_spmd(
        nc,
        _make_in_maps(x, W_query, W_key, W_value),
        core_ids=list(range(NCORES)),
        trace=_trace,
        tmpdir=_tmpdir,
    )
    full = np.empty((B, H, S, D), dtype=np.float32)
    for c in range(NCORES):
        b = c // 2
        hp = (c % 2) * HPC
        full[b, hp:hp + HPC] = res.results[c]["out"]
    if _trace:
        kernel._last_results = res
    return (full, full)
